# revision 16
# baseline (speedup 1.0000x reference)
"""DST-II kernel for Trainium2 (8 NeuronCores, Bass/Tile).

y[m, k] = sum_n x[m, n] * sin(pi/N * (n + 1/2) * (k + 1)),  x: [16384, 1024] f32.

This is a batched matmul y = x @ S with a fixed [1024, 1024] sine table.
Sharding: batch (rows of x) split across 8 cores, S replicated.

Fast-DST folding: S has the row symmetry S[N-1-n, k] = (-1)^k S[n, k], so
with u = x_front + x_back_rev, v = x_front - x_back_rev:
    y[:, 0::2] = u @ A,  A = S[:512, 0::2]          (512x512)
    y[:, 1::2] = v @ B,  B = S[:512, 1::2]          (512x512)
B is itself a DST-II-style kernel with the same symmetry, so the v branch
folds once more (p = fold+(v), q = fold-(v)):
    y[:, 1::4] = p @ B[:256, 0::2],   y[:, 3::4] = q @ B[:256, 1::2]
This removes 3/8 of the matmul FLOPs and 5/8 of the table traffic. All
folds run on the vector engine (split per k-tile so each matmul gates only
on its own slice). A row permutation pi of
the folded space keeps level-2 fold partners partition-aligned; the u-table
rows are permuted identically (contraction is order-invariant).

Implementation notes:
  - TensorE computes out = lhsT.T @ rhs with the contraction dim on
    partitions. The u branch keeps x-derived tiles stationary (output in
    natural row-major orientation). The v branch instead keeps the small
    tables stationary and streams p/q as the moving operand, producing
    512-wide dense matmul streams (TensorE stays HAM-warm) with the output
    transposed; the host merges/transposes the three output blocks.
  - Matmuls run in float32r (TF32-like, ~2 cycles/row for 4-byte operands,
    ~1.8e-4 rel err). Inputs are declared float32r in DRAM directly; the
    hardware accepts raw fp32 bits with accuracy identical to pre-rounded
    data.
  - x is shipped pre-transposed/permuted and packed chunk-contiguously so
    every chunk DMA is one contiguous run per partition; tables ship
    pre-tiled for single-DMA loads. Chunk sizes ramp 128..512..128 to
    shorten the serial head/tail.
  - Loads issue on the Sync HWDGE queue, stores on the GpSimd SWDGE queue,
    and PSUM->SBUF copies run on the Scalar engine, so no engine's FIFO ever
    head-of-line blocks another stage of the pipeline.
"""

import numpy as np
from contextlib import ExitStack

import concourse.bass as bass
import concourse.mybir as mybir
import concourse.tile as tile
from concourse import bacc
from concourse.bass_utils import run_bass_kernel_spmd

N_CORES = 8
B = 16384            # total batch (rows)
N = 1024             # transform length
M_CORE = B // N_CORES   # rows per core = 2048
P = 128
NH = N // 2          # level-1 folded length = 512
NQ = N // 4          # level-2 folded length = 256
CHUNKS = [128, 256, 512, 512, 512, 128]
MAX_CHUNK = max(CHUNKS)
assert sum(CHUNKS) == M_CORE

# permutation of the folded space: tiles [0:128], [128:256], [383:255:-1],
# [511:383:-1] — aligns level-2 fold partners (n', 511-n') across tiles.
PI = np.concatenate([np.arange(0, 128), np.arange(128, 256),
                     np.arange(383, 255, -1), np.arange(511, 383, -1)])

_CACHE = {}


def _dst_table() -> np.ndarray:
    n = np.arange(N, dtype=np.float64)
    k = np.arange(N, dtype=np.float64)
    return np.sin((np.pi / N) * (n[:, None] + 0.5) * (k[None, :] + 1.0))


def _tables():
    S = _dst_table()
    A = S[:NH, 0::2][PI, :].astype(np.float32)          # [512, 512]
    # pack A as [128, 4*512]: per partition, k-tiles side by side
    Ap = np.ascontiguousarray(
        A.reshape(4, P, NH).transpose(1, 0, 2).reshape(P, 4 * NH))
    Bm = S[:NH, 1::2]
    BP = Bm[:NQ, 0::2].astype(np.float32)               # [256, 256]
    BQ = Bm[:NQ, 1::2].astype(np.float32)
    # pack BP,BQ together as [128, 4*256]: order BP0, BP1, BQ0, BQ1
    Bt = np.stack([BP[:P], BP[P:], BQ[:P], BQ[P:]])     # [4, 128, 256]
    Bp = np.ascontiguousarray(Bt.transpose(1, 0, 2).reshape(P, 4 * NQ))
    return Ap, Bp


def _build():
    f32 = mybir.dt.float32
    f32r = mybir.dt.float32r
    nc = bacc.Bacc("TRN2", target_bir_lowering=False, debug=False,
                   enable_asserts=False)
    xP = nc.dram_tensor("xP", [P, 8 * M_CORE], f32r, kind="ExternalInput").ap()
    A = nc.dram_tensor("A", [P, 4 * NH], f32r, kind="ExternalInput").ap()
    Bb = nc.dram_tensor("Bb", [P, 4 * NQ], f32r, kind="ExternalInput").ap()
    # outputs: even columns compact, odd columns transposed (host merges)
    ye = nc.dram_tensor("ye", [M_CORE, NH], f32, kind="ExternalOutput").ap()
    yoT = nc.dram_tensor("yoT", [4 * P, M_CORE], f32, kind="ExternalOutput").ap()

    with tile.TileContext(nc) as tc:
        with ExitStack() as ctx:
            const = ctx.enter_context(tc.tile_pool(name="const", bufs=1))
            xin = ctx.enter_context(tc.tile_pool(name="xin", bufs=5))
            fold = ctx.enter_context(tc.tile_pool(name="fold", bufs=2))
            yeout = ctx.enter_context(tc.tile_pool(name="yeout", bufs=2))
            yoout = ctx.enter_context(tc.tile_pool(name="yoout", bufs=2))
            ps = ctx.enter_context(tc.tile_pool(name="ps", bufs=4, space="PSUM"))

            A_t = const.tile([P, 4, NH], f32r)
            nc.sync.dma_start(A_t[:], A.rearrange("p (o f) -> p o f", o=4))
            B_t = const.tile([P, 4, NQ], f32r)

            m0 = 0
            for ci, mc in enumerate(CHUNKS):
                w = mc
                xc = xin.tile([P, 8 * MAX_CHUNK], f32r, tag="xc")
                nc.sync.dma_start(xc[:, :8 * w], xP[:, 8 * m0:8 * (m0 + w)])
                if ci == 0:
                    nc.sync.dma_start(B_t[:],
                                      Bb.rearrange("p (o f) -> p o f", o=4))
                # per-k fold tiles so a matmul only waits on its own slice
                u = [fold.tile([P, MAX_CHUNK], f32r, tag=f"u{k}", name=f"u{k}")
                     for k in range(4)]
                v = [fold.tile([P, MAX_CHUNK], f32r, tag=f"v{k}", name=f"v{k}")
                     for k in range(4)]
                for k in range(4):
                    nc.vector.tensor_add(u[k][:, :w], xc[:, k * w:(k + 1) * w],
                                         xc[:, (4 + k) * w:(5 + k) * w])
                for k in range(4):
                    nc.vector.tensor_sub(v[k][:, :w], xc[:, k * w:(k + 1) * w],
                                         xc[:, (4 + k) * w:(5 + k) * w])
                pq = [fold.tile([P, MAX_CHUNK], f32r, tag=f"pq{k}", name=f"pq{k}")
                      for k in range(4)]
                nc.vector.tensor_add(pq[0][:, :w], v[0][:, :w], v[3][:, :w])
                nc.vector.tensor_add(pq[1][:, :w], v[1][:, :w], v[2][:, :w])
                nc.vector.tensor_sub(pq[2][:, :w], v[0][:, :w], v[3][:, :w])
                nc.vector.tensor_sub(pq[3][:, :w], v[1][:, :w], v[2][:, :w])

                # u branch: x-tiles stationary, output row-major
                yce = yeout.tile([P, MAX_CHUNK // P, NH], f32, tag="yce")
                for mt in range(mc // P):
                    acc = ps.tile([P, NH], f32, tag="acc_e")
                    for k in range(4):
                        nc.tensor.matmul(
                            acc[:], u[k][:, mt * P:mt * P + P],
                            A_t[:, k, :], start=(k == 0), stop=(k == 3))
                    nc.scalar.copy(out=yce[:, mt, :], in_=acc[:])
                nc.gpsimd.dma_start(
                    ye[m0:m0 + mc, :].rearrange("(o p) f -> p o f", p=P),
                    yce[:, :mc // P, :])

                # v branch: tables stationary, p/q moving, output transposed
                yco = yoout.tile([P, 4, MAX_CHUNK], f32, tag="yco")
                for g in range(4):
                    srcs = (pq[0], pq[1]) if g < 2 else (pq[2], pq[3])
                    acc = ps.tile([P, MAX_CHUNK], f32, tag="acc_o")
                    for k in range(2):
                        nc.tensor.matmul(
                            acc[:, :w],
                            B_t[:, (g & 2) + k, (g & 1) * P:(g & 1) * P + P],
                            srcs[k][:, :w],
                            start=(k == 0), stop=(k == 1))
                    nc.scalar.copy(out=yco[:, g, :w], in_=acc[:, :w])
                nc.gpsimd.dma_start(
                    yoT[:, m0:m0 + mc].rearrange("(o p) f -> p o f", p=P),
                    yco[:, :, :w])
                m0 += mc

    nc.compile()
    return nc


def _get_nc():
    if "nc" not in _CACHE:
        _CACHE["nc"] = _build()
    return _CACHE["nc"]


def _pack_x(xs: np.ndarray) -> np.ndarray:
    """[M_CORE, N] row-slab -> packed [128, 8*M_CORE] fold-ready layout."""
    front = xs[:, PI].T                  # [512, m]
    back = xs[:, 1023 - PI].T            # [512, m]
    xT2 = np.concatenate([front, back], axis=0)   # [1024, m]
    blocks = []
    m0 = 0
    for mc in CHUNKS:
        blk = xT2[:, m0:m0 + mc].reshape(8, P, mc)
        blocks.append(blk.transpose(1, 0, 2).reshape(P, 8 * mc))
        m0 += mc
    return np.ascontiguousarray(np.concatenate(blocks, axis=1))


def _in_maps(x: np.ndarray):
    if "tabs" not in _CACHE:
        _CACHE["tabs"] = _tables()
    Ap, Bp = _CACHE["tabs"]
    x = np.ascontiguousarray(x, dtype=np.float32)
    maps = []
    for c in range(N_CORES):
        xs = x[c * M_CORE:(c + 1) * M_CORE]
        maps.append({"xP": _pack_x(xs), "A": Ap, "Bb": Bp})
    return maps


def _merge(res) -> np.ndarray:
    out = np.empty((B, N), dtype=np.float32)
    for c in range(N_CORES):
        r = res.results[c]
        blk = out[c * M_CORE:(c + 1) * M_CORE]
        blk[:, 0::2] = r["ye"]
        yoT = r["yoT"]                       # [512, M_CORE]: BP0,BP1,BQ0,BQ1
        blk[:, 1::4] = yoT[:2 * P].T
        blk[:, 3::4] = yoT[2 * P:].T
    return out


def kernel(x: np.ndarray) -> np.ndarray:
    nc = _get_nc()
    res = run_bass_kernel_spmd(nc, _in_maps(x), list(range(N_CORES)))
    return _merge(res)


def _install_profile_hooks():
    """The agent image's antenv lacks axon_hooks; recreate it from
    trn_agent_boot so run_bass_kernel_spmd(trace=True) can capture NTFF
    profiles. Also stub out the S3 artifact upload."""
    import sys, types
    import concourse.bass_utils as bu

    if "antenv.axon_hooks" not in sys.modules:
        from trn_agent_boot.trn_boot import _ntff_profile_via_ctypes
        hook = _ntff_profile_via_ctypes("/opt/axon/libaxon_pjrt.so")
        mod = types.ModuleType("antenv.axon_hooks")
        mod.get_axon_ntff_profile_hook = lambda: hook
        mod.set_axon_ntff_profile_hook = lambda h: None
        sys.modules["antenv.axon_hooks"] = mod
    bu.upload_artifacts = lambda tmpdir: f"local:{tmpdir}"


def profile(x: np.ndarray, tmpdir=None, trace_kwargs={}):
    """Run once with NTFF tracing; returns (exec_time_ns, BassKernelResults)."""
    _install_profile_hooks()
    nc = _get_nc()
    res = run_bass_kernel_spmd(nc, _in_maps(x), list(range(N_CORES)),
                               trace=True, tmpdir=tmpdir,
                               trace_kwargs=trace_kwargs)
    return res.exec_time_ns, res


# revision 17
# speedup vs baseline: 1.1654x; 1.1654x over previous
"""DST-II kernel for Trainium2 (8 NeuronCores, Bass/Tile).

y[m, k] = sum_n x[m, n] * sin(pi/N * (n + 1/2) * (k + 1)),  x: [16384, 1024] f32.

This is a batched matmul y = x @ S with a fixed [1024, 1024] sine table.
Sharding: batch (rows of x) split across 8 cores, S replicated.

Fast-DST folding: S has the row symmetry S[N-1-n, k] = (-1)^k S[n, k], so
with u = x_front + x_back_rev, v = x_front - x_back_rev:
    y[:, 0::2] = u @ A,  A = S[:512, 0::2]          (512x512)
    y[:, 1::2] = v @ B,  B = S[:512, 1::2]          (512x512)
B is itself a DST-II-style kernel with the same symmetry, so the v branch
folds once more (p = fold+(v), q = fold-(v)):
    y[:, 1::4] = p @ B[:256, 0::2],   y[:, 3::4] = q @ B[:256, 1::2]
This removes 3/8 of the matmul FLOPs and 5/8 of the table traffic. All
folds run on the vector engine (split per k-tile so each matmul gates only
on its own slice). A row permutation pi of
the folded space keeps level-2 fold partners partition-aligned; the u-table
rows are permuted identically (contraction is order-invariant).

Implementation notes:
  - TensorE computes out = lhsT.T @ rhs with the contraction dim on
    partitions. The u branch keeps x-derived tiles stationary (output in
    natural row-major orientation). The v branch instead keeps the small
    tables stationary and streams p/q as the moving operand, producing
    512-wide dense matmul streams (TensorE stays HAM-warm) with the output
    transposed; the host merges/transposes the three output blocks.
  - Matmuls run in float32r (TF32-like, ~2 cycles/row for 4-byte operands,
    ~1.8e-4 rel err). Inputs are declared float32r in DRAM directly; the
    hardware accepts raw fp32 bits with accuracy identical to pre-rounded
    data.
  - x is shipped pre-transposed/permuted and packed chunk-contiguously so
    every chunk DMA is one contiguous run per partition; tables ship
    pre-tiled for single-DMA loads. Chunk sizes ramp 128..512..128 to
    shorten the serial head/tail.
  - Loads issue on the Sync HWDGE queue, stores on the GpSimd SWDGE queue,
    and PSUM->SBUF copies run on the Scalar engine, so no engine's FIFO ever
    head-of-line blocks another stage of the pipeline.
"""

import numpy as np
from contextlib import ExitStack

import concourse.bass as bass
import concourse.mybir as mybir
import concourse.tile as tile
from concourse import bacc
from concourse.bass_utils import run_bass_kernel_spmd

N_CORES = 8
B = 16384            # total batch (rows)
N = 1024             # transform length
M_CORE = B // N_CORES   # rows per core = 2048
P = 128
NH = N // 2          # level-1 folded length = 512
NQ = N // 4          # level-2 folded length = 256
CHUNKS = [128, 256, 512, 512, 512, 128]
MAX_CHUNK = max(CHUNKS)
assert sum(CHUNKS) == M_CORE

# permutation of the folded space: tiles [0:128], [128:256], [383:255:-1],
# [511:383:-1] — aligns level-2 fold partners (n', 511-n') across tiles.
PI = np.concatenate([np.arange(0, 128), np.arange(128, 256),
                     np.arange(383, 255, -1), np.arange(511, 383, -1)])

_CACHE = {}


def _dst_table() -> np.ndarray:
    n = np.arange(N, dtype=np.float64)
    k = np.arange(N, dtype=np.float64)
    return np.sin((np.pi / N) * (n[:, None] + 0.5) * (k[None, :] + 1.0))


def _tables():
    S = _dst_table()
    A = S[:NH, 0::2][PI, :].astype(np.float32)          # [512, 512]
    # pack A as [128, 4*512]: per partition, k-tiles side by side
    Ap = np.ascontiguousarray(
        A.reshape(4, P, NH).transpose(1, 0, 2).reshape(P, 4 * NH))
    Bm = S[:NH, 1::2]
    BP = Bm[:NQ, 0::2].astype(np.float32)               # [256, 256]
    BQ = Bm[:NQ, 1::2].astype(np.float32)
    # pack BP,BQ together as [128, 4*256]: order BP0, BP1, BQ0, BQ1
    Bt = np.stack([BP[:P], BP[P:], BQ[:P], BQ[P:]])     # [4, 128, 256]
    Bp = np.ascontiguousarray(Bt.transpose(1, 0, 2).reshape(P, 4 * NQ))
    return Ap, Bp


def _build():
    f32 = mybir.dt.float32
    f32r = mybir.dt.float32r
    nc = bacc.Bacc("TRN2", target_bir_lowering=False, debug=False,
                   enable_asserts=False)
    xP = nc.dram_tensor("xP", [P, 8 * M_CORE], f32r, kind="ExternalInput").ap()
    A = nc.dram_tensor("A", [P, 4 * NH], f32r, kind="ExternalInput").ap()
    Bb = nc.dram_tensor("Bb", [P, 4 * NQ], f32r, kind="ExternalInput").ap()
    # outputs: even columns compact, odd columns transposed (host merges)
    ye = nc.dram_tensor("ye", [M_CORE, NH], f32, kind="ExternalOutput").ap()
    yoT = nc.dram_tensor("yoT", [4 * P, M_CORE], f32, kind="ExternalOutput").ap()

    with tile.TileContext(nc) as tc:
        with ExitStack() as ctx:
            const = ctx.enter_context(tc.tile_pool(name="const", bufs=1))
            xin = ctx.enter_context(tc.tile_pool(name="xin", bufs=4))
            fold = ctx.enter_context(tc.tile_pool(name="fold", bufs=2))
            yeout = ctx.enter_context(tc.tile_pool(name="yeout", bufs=2))
            yoout = ctx.enter_context(tc.tile_pool(name="yoout", bufs=2))
            ps = ctx.enter_context(tc.tile_pool(name="ps", bufs=3, space="PSUM"))

            A_t = const.tile([P, 4, NH], f32r)
            nc.sync.dma_start(A_t[:], A.rearrange("p (o f) -> p o f", o=4))
            B_t = const.tile([P, 4, NQ], f32r)

            m0 = 0
            for ci, mc in enumerate(CHUNKS):
                w = mc
                xc = xin.tile([P, 8 * MAX_CHUNK], f32r, tag="xc")
                nc.sync.dma_start(xc[:, :8 * w], xP[:, 8 * m0:8 * (m0 + w)])
                if ci == 0:
                    nc.sync.dma_start(B_t[:],
                                      Bb.rearrange("p (o f) -> p o f", o=4))
                # per-k fold tiles so a matmul only waits on its own slice
                u = [fold.tile([P, MAX_CHUNK], f32r, tag=f"u{k}", name=f"u{k}")
                     for k in range(4)]
                v = [fold.tile([P, MAX_CHUNK], f32r, tag=f"v{k}", name=f"v{k}")
                     for k in range(4)]
                for k in range(4):
                    nc.vector.tensor_add(u[k][:, :w], xc[:, k * w:(k + 1) * w],
                                         xc[:, (4 + k) * w:(5 + k) * w])
                for k in range(4):
                    nc.vector.tensor_sub(v[k][:, :w], xc[:, k * w:(k + 1) * w],
                                         xc[:, (4 + k) * w:(5 + k) * w])
                pq = [fold.tile([P, MAX_CHUNK], f32r, tag=f"pq{k}", name=f"pq{k}")
                      for k in range(4)]
                nc.vector.tensor_add(pq[0][:, :w], v[0][:, :w], v[3][:, :w])
                nc.vector.tensor_add(pq[1][:, :w], v[1][:, :w], v[2][:, :w])
                nc.vector.tensor_sub(pq[2][:, :w], v[0][:, :w], v[3][:, :w])
                nc.vector.tensor_sub(pq[3][:, :w], v[1][:, :w], v[2][:, :w])

                # u branch: x-tiles stationary, output row-major
                yce = yeout.tile([P, MAX_CHUNK // P, NH], f32, tag="yce")
                for mt in range(mc // P):
                    acc = ps.tile([P, NH], f32, tag="acc_e")
                    for k in range(4):
                        nc.tensor.matmul(
                            acc[:], u[k][:, mt * P:mt * P + P],
                            A_t[:, k, :], start=(k == 0), stop=(k == 3))
                    nc.scalar.copy(out=yce[:, mt, :], in_=acc[:])
                nc.gpsimd.dma_start(
                    ye[m0:m0 + mc, :].rearrange("(o p) f -> p o f", p=P),
                    yce[:, :mc // P, :])

                # v branch: tables stationary, p/q moving, output transposed
                yco = yoout.tile([P, 4, MAX_CHUNK], f32, tag="yco")
                for g in range(4):
                    srcs = (pq[0], pq[1]) if g < 2 else (pq[2], pq[3])
                    acc = ps.tile([P, MAX_CHUNK], f32, tag="acc_o")
                    for k in range(2):
                        nc.tensor.matmul(
                            acc[:, :w],
                            B_t[:, (g & 2) + k, (g & 1) * P:(g & 1) * P + P],
                            srcs[k][:, :w],
                            start=(k == 0), stop=(k == 1))
                    nc.scalar.copy(out=yco[:, g, :w], in_=acc[:, :w])
                nc.gpsimd.dma_start(
                    yoT[:, m0:m0 + mc].rearrange("(o p) f -> p o f", p=P),
                    yco[:, :, :w])
                m0 += mc

    nc.compile()
    return nc


def _get_nc():
    if "nc" not in _CACHE:
        _CACHE["nc"] = _build()
    return _CACHE["nc"]


def _pack_x(xs: np.ndarray) -> np.ndarray:
    """[M_CORE, N] row-slab -> packed [128, 8*M_CORE] fold-ready layout."""
    front = xs[:, PI].T                  # [512, m]
    back = xs[:, 1023 - PI].T            # [512, m]
    xT2 = np.concatenate([front, back], axis=0)   # [1024, m]
    blocks = []
    m0 = 0
    for mc in CHUNKS:
        blk = xT2[:, m0:m0 + mc].reshape(8, P, mc)
        blocks.append(blk.transpose(1, 0, 2).reshape(P, 8 * mc))
        m0 += mc
    return np.ascontiguousarray(np.concatenate(blocks, axis=1))


def _in_maps(x: np.ndarray):
    if "tabs" not in _CACHE:
        _CACHE["tabs"] = _tables()
    Ap, Bp = _CACHE["tabs"]
    x = np.ascontiguousarray(x, dtype=np.float32)
    maps = []
    for c in range(N_CORES):
        xs = x[c * M_CORE:(c + 1) * M_CORE]
        maps.append({"xP": _pack_x(xs), "A": Ap, "Bb": Bp})
    return maps


def _merge(res) -> np.ndarray:
    out = np.empty((B, N), dtype=np.float32)
    for c in range(N_CORES):
        r = res.results[c]
        blk = out[c * M_CORE:(c + 1) * M_CORE]
        blk[:, 0::2] = r["ye"]
        yoT = r["yoT"]                       # [512, M_CORE]: BP0,BP1,BQ0,BQ1
        blk[:, 1::4] = yoT[:2 * P].T
        blk[:, 3::4] = yoT[2 * P:].T
    return out


def kernel(x: np.ndarray) -> np.ndarray:
    nc = _get_nc()
    res = run_bass_kernel_spmd(nc, _in_maps(x), list(range(N_CORES)))
    return _merge(res)


def _install_profile_hooks():
    """The agent image's antenv lacks axon_hooks; recreate it from
    trn_agent_boot so run_bass_kernel_spmd(trace=True) can capture NTFF
    profiles. Also stub out the S3 artifact upload."""
    import sys, types
    import concourse.bass_utils as bu

    if "antenv.axon_hooks" not in sys.modules:
        from trn_agent_boot.trn_boot import _ntff_profile_via_ctypes
        hook = _ntff_profile_via_ctypes("/opt/axon/libaxon_pjrt.so")
        mod = types.ModuleType("antenv.axon_hooks")
        mod.get_axon_ntff_profile_hook = lambda: hook
        mod.set_axon_ntff_profile_hook = lambda h: None
        sys.modules["antenv.axon_hooks"] = mod
    bu.upload_artifacts = lambda tmpdir: f"local:{tmpdir}"


def profile(x: np.ndarray, tmpdir=None, trace_kwargs={}):
    """Run once with NTFF tracing; returns (exec_time_ns, BassKernelResults)."""
    _install_profile_hooks()
    nc = _get_nc()
    res = run_bass_kernel_spmd(nc, _in_maps(x), list(range(N_CORES)),
                               trace=True, tmpdir=tmpdir,
                               trace_kwargs=trace_kwargs)
    return res.exec_time_ns, res
